# revision 1
# baseline (speedup 1.0000x reference)
"""Trainium2 Bass kernel for nn_DA3CrossFrameCFDistanceLoss.

Strategy (8 NeuronCores):
  Phase 1 (data-parallel over batch x extra-frame shard):
    core c -> (b = c//4, shard s = c%4).  Each core streams one teacher
    extra frame transposed (extT [D, 4096] = teacher[b, EXTRA_FRAMES[s]].T),
    computes cosine-similarity sim[r, e] = (ref_n . x_e) * rinv[e] with the
    TensorEngine (float32r matmuls, fp32 PSUM accumulation), and extracts the
    per-shard top-8 values + indices per ref row with the DVE max/max_index
    instructions.  Host merges the 4 shards' top-8 per row into the global
    top-4 (the cross-shard topk merge) and gathers the selected rows.
  Phase 2 (data-parallel over (batch, row-half, feature-half)):
    core c -> (b, h, dh).  Each core computes, for its 128 ref rows and 512
    feature columns, the 19 KL "units":
        d1 (j=0..2):  xt = ref_t - shared_t[j],   xs = ref_s - shared_s[j]
        d2 (k=0..3):  xt = ref_t - sim_high[k],   xs = ref_s - sim_high[k]
        d3 (j,k):     xt = shared_t[j] - simh[k], xs = shared_s[j] - simh[k]
    For each unit it produces partial (Zt, Zs, num) where
        Zt = sum exp(xt), Zs = sum exp(xs), num = sum exp(xt) * (xt - xs)
    using fused ACT exp+accum and DVE tensor_tensor_reduce.  Host combines
    the two feature-halves, evaluates kl = num/Zt - log Zt + log Zs, applies
    SmoothL1 and the weighted averaging.
"""

import numpy as np

import concourse.bass as bass
from concourse import bacc
import concourse.mybir as mybir
from concourse import bass_utils
from concourse.tile import TileContext

# ---- problem constants (hardcoded from the nn.Module defaults) ----
B, V, P, D = 2, 8, 4096, 1024
EXTRA_FRAMES = [1, 3, 5, 7]
SHARED_TEACHER = [2, 4, 6]
SHARED_STUDENT = [1, 2, 3]
NUM_REF = 256
NUM_SHARED = 256
TOPK = 4
TEMP = 1.0
BETA = 0.5
N_CORES = 8

ES = P          # extra rows per shard (one frame per shard)
EB = 1024       # phase-1 e-block size
NBLK = ES // EB
DH = D // 2     # phase-2 feature half
N_UNITS = 19    # 3 d1 + 4 d2 + 12 d3

F32 = mybir.dt.float32
F32R = mybir.dt.float32r
U32 = mybir.dt.uint32

_CACHE = {}

# Results of the most recent launches (exec_time_ns etc), for test harnesses.
LAST_PERF = {}


def _build_phase1():
    nc = bacc.Bacc("TRN2", target_bir_lowering=False, debug=False,
                   enable_asserts=False, num_devices=N_CORES)
    extT = nc.dram_tensor("extT", (D, ES), F32R, kind="ExternalInput").ap()
    refT = nc.dram_tensor("refT", (D, NUM_REF), F32R, kind="ExternalInput").ap()
    rinv = nc.dram_tensor("rinv", (1, ES), F32, kind="ExternalInput").ap()
    vals_o = nc.dram_tensor("vals", (2, 128, 8), F32, kind="ExternalOutput").ap()
    idx_o = nc.dram_tensor("idx", (2, 128, 8), U32, kind="ExternalOutput").ap()

    extT_r = extT.rearrange("(k p) e -> p k e", p=128)
    refT_r = refT.rearrange("(k p) r -> p k r", p=128)

    with TileContext(nc) as tc:
        with (
            tc.tile_pool(name="const", bufs=1) as const_pool,
            tc.tile_pool(name="xin", bufs=3) as xin_pool,
            tc.tile_pool(name="ps", bufs=3, space="PSUM") as ps_pool,
            tc.tile_pool(name="small", bufs=1) as small_pool,
        ):
            refT_sb = const_pool.tile([128, 8, NUM_REF], F32R)
            nc.sync.dma_start(out=refT_sb, in_=refT_r)
            rinv_rep = const_pool.tile([128, ES], F32)
            nc.sync.dma_start(out=rinv_rep, in_=rinv.to_broadcast((128, ES)))
            sim_sb = const_pool.tile([128, 2, ES], F32)
            bv = small_pool.tile([128, 2, NBLK, 8], F32)
            fv = small_pool.tile([128, 2, 8], F32)
            fidx = small_pool.tile([128, 2, 8], U32)

            for eb in range(NBLK):
                esl = slice(eb * EB, (eb + 1) * EB)
                xt = xin_pool.tile([128, 8, EB], F32R, tag="xt")
                nc.sync.dma_start(out=xt, in_=extT_r[:, :, esl])
                # A PE matmul may carry at most one semaphore wait (walrus
                # S3_LW limit).  Consume the xt-DMA dependency with a throwaway
                # matmul so the real matmuls only ever wait on one source.
                dum = ps_pool.tile([128, 512], F32, tag="dum", name="dum", bufs=1)
                nc.tensor.matmul(dum, lhsT=xt[:, 0, 0:128], rhs=xt[:, 0, 0:512],
                                 start=True, stop=True, skip_group_check=True)
                for m in range(2):
                    ps = ps_pool.tile([128, EB], F32, tag="ps", name="ps")
                    for k in range(8):
                        for nn in range(EB // 512):
                            nc.tensor.matmul(
                                ps[:, nn * 512:(nn + 1) * 512],
                                lhsT=refT_sb[:, k, m * 128:(m + 1) * 128],
                                rhs=xt[:, k, nn * 512:(nn + 1) * 512],
                                start=(k == 0), stop=(k == 7),
                            )
                    nc.vector.tensor_mul(sim_sb[:, m, esl], ps, rinv_rep[:, esl])
                    nc.vector.max(out=bv[:, m, eb, :], in_=sim_sb[:, m, esl])
            for m in range(2):
                nc.vector.max(out=fv[:, m, :], in_=bv[:, m, :, :])
                nc.vector.max_index(out=fidx[:, m, :], in_max=fv[:, m, :],
                                    in_values=sim_sb[:, m, :])
                nc.sync.dma_start(out=vals_o[m], in_=fv[:, m, :])
                nc.sync.dma_start(out=idx_o[m], in_=fidx[:, m, :])
    nc.compile()
    return nc


def _phase2_units():
    units = [("d1", j, None) for j in range(3)]
    units += [("d2", None, k) for k in range(4)]
    units += [("d3", j, k) for j in range(3) for k in range(4)]
    return units


def _build_phase2():
    nc = bacc.Bacc("TRN2", target_bir_lowering=False, debug=False,
                   enable_asserts=False, num_devices=N_CORES)
    REF = nc.dram_tensor("ref2", (2, 128, DH), F32, kind="ExternalInput").ap()
    SHT = nc.dram_tensor("sht", (3, 128, DH), F32, kind="ExternalInput").ap()
    SHS = nc.dram_tensor("shs", (3, 128, DH), F32, kind="ExternalInput").ap()
    SIMH = nc.dram_tensor("simh", (4, 128, DH), F32, kind="ExternalInput").ap()
    ZB_o = nc.dram_tensor("zb", (128, N_UNITS, 3), F32, kind="ExternalOutput").ap()

    Exp = mybir.ActivationFunctionType.Exp
    mult = mybir.AluOpType.mult
    add = mybir.AluOpType.add

    with TileContext(nc) as tc:
        with (
            tc.tile_pool(name="src", bufs=1) as src_pool,
            tc.tile_pool(name="work", bufs=3) as work_pool,
        ):
            ref_sb = src_pool.tile([128, 2, DH], F32)
            nc.sync.dma_start(out=ref_sb, in_=REF.rearrange("a p d -> p a d"))
            sht_sb = src_pool.tile([128, 3, DH], F32)
            nc.sync.dma_start(out=sht_sb, in_=SHT.rearrange("a p d -> p a d"))
            shs_sb = src_pool.tile([128, 3, DH], F32)
            nc.sync.dma_start(out=shs_sb, in_=SHS.rearrange("a p d -> p a d"))
            simh_sb = src_pool.tile([128, 4, DH], F32)
            nc.sync.dma_start(out=simh_sb, in_=SIMH.rearrange("a p d -> p a d"))

            zb = src_pool.tile([128, N_UNITS, 3], F32)

            # shared difference tensors: d = xt - xs per unit type
            rd = src_pool.tile([128, DH], F32)
            nc.vector.tensor_sub(rd, ref_sb[:, 0, :], ref_sb[:, 1, :])
            sd = src_pool.tile([128, 3, DH], F32)
            dd1 = src_pool.tile([128, 3, DH], F32)
            for j in range(3):
                nc.gpsimd.tensor_sub(sd[:, j, :], sht_sb[:, j, :], shs_sb[:, j, :])
                nc.vector.tensor_sub(dd1[:, j, :], rd, sd[:, j, :])

            for u, (typ, j, k) in enumerate(_phase2_units()):
                if typ == "d1":
                    At, Bt = ref_sb[:, 0, :], sht_sb[:, j, :]
                    As, Bs = ref_sb[:, 1, :], shs_sb[:, j, :]
                    dap = dd1[:, j, :]
                elif typ == "d2":
                    At, Bt = ref_sb[:, 0, :], simh_sb[:, k, :]
                    As, Bs = ref_sb[:, 1, :], simh_sb[:, k, :]
                    dap = rd
                else:
                    At, Bt = sht_sb[:, j, :], simh_sb[:, k, :]
                    As, Bs = shs_sb[:, j, :], simh_sb[:, k, :]
                    dap = sd[:, j, :]

                xt = work_pool.tile([128, DH], F32, tag="xt", name="xt")
                nc.gpsimd.tensor_sub(xt, At, Bt)
                xs = work_pool.tile([128, DH], F32, tag="xs", name="xs")
                nc.vector.tensor_sub(xs, As, Bs)
                et = work_pool.tile([128, DH], F32, tag="et", name="et")
                nc.scalar.activation(et, xt, Exp, accum_out=zb[:, u, 0:1])
                es = work_pool.tile([128, DH], F32, tag="es", name="es")
                nc.scalar.activation(es, xs, Exp, accum_out=zb[:, u, 1:2])
                w = work_pool.tile([128, DH], F32, tag="w", name="w")
                nc.vector.scalar_tensor_tensor(
                    out=w, in0=et, scalar=1.0, in1=dap,
                    op0=mult, op1=mult, accum_out=zb[:, u, 2:3],
                )

            nc.sync.dma_start(out=ZB_o, in_=zb)
    nc.compile()
    return nc


def _get(name):
    if name not in _CACHE:
        _CACHE[name] = _build_phase1() if name == "p1" else _build_phase2()
    return _CACHE[name]


def kernel(**inputs):
    tf = np.ascontiguousarray(np.asarray(inputs["teacher_feats"], dtype=np.float32))
    sf = np.ascontiguousarray(np.asarray(inputs["student_feats"], dtype=np.float32))
    in_dtype = np.asarray(inputs["ref_perm"]).dtype
    ref_perm = np.asarray(inputs["ref_perm"]).astype(np.int64)[:NUM_REF]
    shared_perm = np.asarray(inputs["shared_perm"]).astype(np.int64)[:NUM_SHARED]
    assert in_dtype == np.int32

    # ---- host gathers + ref normalization (tiny) ----
    ref_t = tf[:, 0, ref_perm, :]                       # [B, 256, 1024]
    ref_s = sf[:, 0, ref_perm, :]
    rn = np.sqrt(np.einsum("brd,brd->br", ref_t, ref_t))[..., None]
    refn = ref_t / np.maximum(rn, 1e-12)
    refTs = [np.ascontiguousarray(refn[b].T) for b in range(B)]

    # ---- phase 1: sharded cosine-sim + per-shard top-8 ----
    in_maps1 = []
    for c in range(N_CORES):
        b, s = divmod(c, 4)
        x = tf[b, EXTRA_FRAMES[s]]                      # [4096, 1024]
        extT = np.ascontiguousarray(x.T)                # [1024, 4096]
        nrm = np.sqrt(np.einsum("ed,ed->e", x, x))
        rinv = (1.0 / np.maximum(nrm, 1e-12)).astype(np.float32)[None, :]
        in_maps1.append({"extT": extT, "refT": refTs[b], "rinv": rinv})

    res1 = bass_utils.run_bass_kernel_spmd(
        _get("p1"), in_maps1, core_ids=list(range(N_CORES)))
    LAST_PERF["p1"] = res1

    # ---- host cross-shard top-k merge ----
    gidx = np.zeros((B, NUM_REF, TOPK), dtype=np.int64)
    for b in range(B):
        vals = np.concatenate(
            [res1.results[b * 4 + s]["vals"].reshape(NUM_REF, 8)
             for s in range(4)], axis=1)                # [256, 32]
        idxs = np.concatenate(
            [res1.results[b * 4 + s]["idx"].reshape(NUM_REF, 8).astype(np.int64)
             + s * ES for s in range(4)], axis=1)
        order = np.argsort(-vals, axis=1, kind="stable")[:, :TOPK]
        gidx[b] = np.take_along_axis(idxs, order, axis=1)

    fr = np.asarray(EXTRA_FRAMES, dtype=np.int64)[gidx // P]
    pt = gidx % P
    sim_high = tf[np.arange(B)[:, None, None], fr, pt]  # [B, 256, 4, 1024]

    # ---- phase 2: distances ----
    sh_t = np.stack([tf[:, t, shared_perm, :] for t in SHARED_TEACHER], axis=1)
    sh_s = np.stack([sf[:, s, shared_perm, :] for s in SHARED_STUDENT], axis=1)

    in_maps2 = []
    for c in range(N_CORES):
        b, h, dh = c >> 2, (c >> 1) & 1, c & 1
        rs = slice(h * 128, (h + 1) * 128)
        cs = slice(dh * DH, (dh + 1) * DH)
        ref2 = np.ascontiguousarray(
            np.stack([ref_t[b, rs, cs], ref_s[b, rs, cs]]))
        sht = np.ascontiguousarray(sh_t[b, :, rs, cs])
        shs = np.ascontiguousarray(sh_s[b, :, rs, cs])
        simh = np.ascontiguousarray(sim_high[b, rs, :, cs].transpose(1, 0, 2))
        in_maps2.append({"ref2": ref2, "sht": sht, "shs": shs, "simh": simh})

    res2 = bass_utils.run_bass_kernel_spmd(
        _get("p2"), in_maps2, core_ids=list(range(N_CORES)))
    LAST_PERF["p2"] = res2

    # ---- host tail: kl + SmoothL1 + averaging ----
    s1 = s2 = s3 = 0.0
    for b in range(B):
        for h in range(2):
            z = (res2.results[b * 4 + h * 2 + 0]["zb"].astype(np.float64)
                 + res2.results[b * 4 + h * 2 + 1]["zb"].astype(np.float64))
            Zt, Zs, num = z[..., 0], z[..., 1], z[..., 2]   # [128, 19]
            kl = num / Zt - np.log(Zt) + np.log(Zs)
            akl = np.abs(kl)
            hub = np.where(akl < BETA, 0.5 * kl * kl / BETA, akl - 0.5 * BETA)
            s1 += hub[:, 0:3].sum()
            s2 += hub[:, 3:7].sum()
            s3 += hub[:, 7:19].sum()

    loss = (s1 / (3 * B * NUM_REF)
            + s2 / (B * NUM_REF * TOPK)
            + s3 / (3 * B * NUM_REF * TOPK))
    return np.float32(loss)



# revision 8
# speedup vs baseline: 1.3667x; 1.3667x over previous
"""Trainium2 Bass kernel for nn_DA3CrossFrameCFDistanceLoss.

Strategy (8 NeuronCores):
  Phase 1 (data-parallel over batch x extra-frame shard):
    core c -> (b = c//4, shard s = c%4).  Host pre-normalizes the ref rows
    and the shard's candidate rows and quantizes both to fp8e4m3 (fp16
    fallback), packed partition-major so every DMA descriptor is an 8KB
    contiguous run.  The PE computes cosine sims with DoubleRow fp8
    matmuls (2 k-chunks per instruction), ACT copies each finished PSUM
    block to SBUF as fp16, and the DVE extracts per-1024-block top-8
    values + indices, pipelined behind the matmuls.  Host merges the
    4 shards x 4 blocks x 8 candidates per row into the global top-4.
  Phase 2 (data-parallel over (batch, row-half, feature-half)):
    exp(a-b) = exp(a)*exp(-b): ACT precomputes the 18 per-source exps in
    4 fused ops; each of the 19 KL units then needs only 3 DVE
    tensor_tensor_reduce ops (Zt, Zs, num) over fp16 operands, with a few
    Zs reductions diverted to GpSimd.  Host combines the feature-half
    partials, evaluates kl = num/Zt - log Zt + log Zs, SmoothL1, and the
    weighted averaging.
"""

import os

import numpy as np
import ml_dtypes

import concourse.bass as bass
from concourse import bacc
import concourse.mybir as mybir
from concourse import bass_utils
from concourse.tile import TileContext

# ---- problem constants (hardcoded from the nn.Module defaults) ----
B, V, P, D = 2, 8, 4096, 1024
EXTRA_FRAMES = [1, 3, 5, 7]
SHARED_TEACHER = [2, 4, 6]
SHARED_STUDENT = [1, 2, 3]
NUM_REF = 256
NUM_SHARED = 256
TOPK = 4
BETA = 0.5
N_CORES = 8

EB = 1024                 # phase-1 e-block size
NBLK = P // EB            # 4 blocks per shard
DH = D // 2               # phase-2 feature half
N_UNITS = 19              # 3 d1 + 4 d2 + 12 d3

P1_DT = os.environ.get("BASS_P1_DT", "fp8")   # "fp8" | "fp16"
NG = int(os.environ.get("BASS_P2_NG", "0"))   # d3-Zs ops on gpsimd

F32 = mybir.dt.float32
F16 = mybir.dt.float16
F8 = mybir.dt.float8e4
U16 = mybir.dt.uint16

_CACHE = {}

# Results of the most recent launches (exec_time_ns etc), for test harnesses.
LAST_PERF = {}


def _build_phase1():
    DT = F8 if P1_DT == "fp8" else F16
    nc = bacc.Bacc("TRN2", target_bir_lowering=False, debug=False,
                   enable_asserts=False, num_devices=N_CORES)
    refP = nc.dram_tensor("refP", (128, 8, NUM_REF), DT, kind="ExternalInput").ap()
    extP = nc.dram_tensor("extP", (128, NBLK, 8, EB), DT, kind="ExternalInput").ap()
    vals_o = nc.dram_tensor("vals", (NBLK, 2, 128, 8), F16, kind="ExternalOutput").ap()
    idx_o = nc.dram_tensor("idx", (NBLK, 2, 128, 8), U16, kind="ExternalOutput").ap()

    DR = mybir.MatmulPerfMode.DoubleRow

    with TileContext(nc) as tc:
        with (
            tc.tile_pool(name="const", bufs=1) as cpool,
            tc.tile_pool(name="xin", bufs=2) as xpool,
            tc.tile_pool(name="sim", bufs=3) as spool,
            tc.tile_pool(name="ps", bufs=3, space="PSUM") as pspool,
            tc.tile_pool(name="out", bufs=4) as opool,
        ):
            ref_sb = cpool.tile([128, 8, NUM_REF], DT)
            nc.sync.dma_start(out=ref_sb, in_=refP)
            for eb in range(NBLK):
                xt = xpool.tile([128, 8, EB], DT, tag="xt")
                nc.sync.dma_start(out=xt, in_=extP[:, eb])
                # A PE matmul may carry at most one semaphore wait (walrus
                # S3_LW limit).  Consume the xt-DMA dependency with a
                # throwaway matmul so the real matmuls only wait on PSUM.
                dum = pspool.tile([128, 512], F32, tag="dum", name="dum", bufs=1)
                nc.tensor.matmul(dum, lhsT=xt[:, 0, 0:128], rhs=xt[:, 0, 0:512],
                                 start=True, stop=True, skip_group_check=True)
                for m in range(2):
                    ps = pspool.tile([128, EB], F32, tag="ps", name="ps")
                    msl = slice(m * 128, (m + 1) * 128)
                    if DT == F8:
                        for kk in range(4):
                            for nn in range(EB // 512):
                                nc.tensor.matmul(
                                    ps[:, nn * 512:(nn + 1) * 512],
                                    lhsT=ref_sb[:, 2 * kk:2 * kk + 2, msl],
                                    rhs=xt[:, 2 * kk:2 * kk + 2,
                                           nn * 512:(nn + 1) * 512],
                                    start=(kk == 0), stop=(kk == 3),
                                    perf_mode=DR,
                                )
                    else:
                        for k in range(8):
                            for nn in range(EB // 512):
                                nc.tensor.matmul(
                                    ps[:, nn * 512:(nn + 1) * 512],
                                    lhsT=ref_sb[:, k, msl],
                                    rhs=xt[:, k, nn * 512:(nn + 1) * 512],
                                    start=(k == 0), stop=(k == 7),
                                )
                    sim = spool.tile([128, EB], F16, tag="sim", name="sim")
                    nc.scalar.copy(sim, ps)
                    mv = opool.tile([128, 8], F16, tag="mv", name="mv")
                    nc.vector.max(mv, sim)
                    mi = opool.tile([128, 8], U16, tag="mi", name="mi")
                    nc.vector.max_index(mi, mv, sim)
                    nc.sync.dma_start(out=vals_o[eb, m], in_=mv)
                    nc.sync.dma_start(out=idx_o[eb, m], in_=mi)
    nc.compile()
    return nc


def _p2_unit_order():
    """Emission order of the 19 units: d2 first, then d3, then d1.
    Returns list of (u, kind, j, k); u is the reference unit index
    (d1 j -> u=j, d2 k -> u=3+k, d3 (j,k) -> u=7+4j+k)."""
    order = [(3 + k, "d2", None, k) for k in range(4)]
    order += [(7 + 4 * j + k, "d3", j, k) for j in range(3) for k in range(4)]
    order += [(j, "d1", j, None) for j in range(3)]
    return order


def _p2_colmap(ng):
    """Map (u, comp) -> ('v'|'g', col).  comp: 0=Zt, 1=Zs, 2=num.
    The last `ng` d3 units' Zs reductions go to gpsimd."""
    cols = {}
    vi = gi = 0
    for u, kind, j, k in _p2_unit_order():
        for c in range(3):
            if c == 1 and kind == "d3" and (11 - (4 * j + k)) < ng:
                cols[(u, c)] = ("g", gi)
                gi += 1
            else:
                cols[(u, c)] = ("v", vi)
                vi += 1
    return cols, vi, gi


def _build_phase2():
    cols, nv, ng = _p2_colmap(NG)
    nc = bacc.Bacc("TRN2", target_bir_lowering=False, debug=False,
                   enable_asserts=False, num_devices=N_CORES)
    SRC = nc.dram_tensor("src", (128, 12, DH), F16, kind="ExternalInput").ap()
    ZV = nc.dram_tensor("zv", (128, nv), F32, kind="ExternalOutput").ap()
    ZG = (nc.dram_tensor("zg", (128, ng), F32, kind="ExternalOutput").ap()
          if ng else None)

    Exp = mybir.ActivationFunctionType.Exp
    mult = mybir.AluOpType.mult
    add = mybir.AluOpType.add

    with TileContext(nc) as tc:
        with tc.tile_pool(name="main", bufs=1) as pool:
            src = pool.tile([128, 12, DH], F16)
            nc.sync.dma_start(out=src, in_=SRC)
            # src slots: 0=ref_t 1=ref_s 2..4=sht_j 5..7=shs_j 8..11=simh_k
            epos = pool.tile([128, 8, DH], F16)   # exp(+src[0:8])
            eneg = pool.tile([128, 10, DH], F16)  # 0..2=-sht 3..5=-shs 6..9=-simh
            ets = pool.tile([128, N_UNITS, DH], F16)
            scr = pool.tile([128, DH], F16)
            gscr = pool.tile([128, DH], F16) if ng else None
            rd = pool.tile([128, DH], F16)
            sd = pool.tile([128, 3, DH], F16)
            dd1 = pool.tile([128, 3, DH], F16)
            zv = pool.tile([128, nv], F32)
            zg = pool.tile([128, ng], F32) if ng else None

            # difference tensors (DVE, can start as soon as src lands)
            nc.vector.tensor_sub(rd, src[:, 0, :], src[:, 1, :])
            nc.vector.tensor_sub(sd, src[:, 2:5, :], src[:, 5:8, :])
            for j in range(3):
                nc.vector.tensor_sub(dd1[:, j, :], rd, sd[:, j, :])

            # fused exps (ACT), ordered so d2/d3 deps resolve first
            nc.scalar.activation(epos[:, 0:2, :], src[:, 0:2, :], Exp)
            nc.scalar.activation(eneg[:, 6:10, :], src[:, 8:12, :], Exp, scale=-1.0)
            nc.scalar.activation(epos[:, 2:8, :], src[:, 2:8, :], Exp)
            nc.scalar.activation(eneg[:, 0:6, :], src[:, 2:8, :], Exp, scale=-1.0)

            def vttr(out, a, b_, u, c):
                kind, i = cols[(u, c)]
                assert kind == "v"
                nc.vector.scalar_tensor_tensor(
                    out=out, in0=a, scalar=1.0, in1=b_,
                    op0=mult, op1=mult, accum_out=zv[:, i:i + 1])

            for u, kind, j, k in _p2_unit_order():
                if kind == "d2":
                    vttr(ets[:, u, :], epos[:, 0, :], eneg[:, 6 + k, :], u, 0)
                    vttr(scr, epos[:, 1, :], eneg[:, 6 + k, :], u, 1)
                    vttr(scr, ets[:, u, :], rd, u, 2)
                elif kind == "d3":
                    vttr(ets[:, u, :], epos[:, 2 + j, :], eneg[:, 6 + k, :], u, 0)
                    ckind, gi = cols[(u, 1)]
                    if ckind == "g":
                        nc.gpsimd.scalar_tensor_tensor(
                            out=gscr, in0=epos[:, 5 + j, :], scalar=1.0,
                            in1=eneg[:, 6 + k, :], op0=mult, op1=mult,
                            accum_out=zg[:, gi:gi + 1])
                    else:
                        vttr(scr, epos[:, 5 + j, :], eneg[:, 6 + k, :], u, 1)
                    vttr(scr, ets[:, u, :], sd[:, j, :], u, 2)
                else:  # d1
                    vttr(ets[:, u, :], epos[:, 0, :], eneg[:, j, :], u, 0)
                    vttr(scr, epos[:, 1, :], eneg[:, 3 + j, :], u, 1)
                    vttr(scr, ets[:, u, :], dd1[:, j, :], u, 2)

            nc.sync.dma_start(out=ZV, in_=zv)
            if ng:
                nc.sync.dma_start(out=ZG, in_=zg)
    nc.compile()
    return nc, cols, nv, ng


def _get(name):
    if name not in _CACHE:
        _CACHE[name] = _build_phase1() if name == "p1" else _build_phase2()
    return _CACHE[name]


def _norm_rows(x):
    n = np.sqrt(np.einsum("...d,...d->...", x, x))
    return x / np.maximum(n, 1e-12)[..., None]


def kernel(**inputs):
    tf = np.ascontiguousarray(np.asarray(inputs["teacher_feats"], dtype=np.float32))
    sf = np.ascontiguousarray(np.asarray(inputs["student_feats"], dtype=np.float32))
    in_dtype = np.asarray(inputs["ref_perm"]).dtype
    ref_perm = np.asarray(inputs["ref_perm"]).astype(np.int64)[:NUM_REF]
    shared_perm = np.asarray(inputs["shared_perm"]).astype(np.int64)[:NUM_SHARED]
    assert in_dtype == np.int32

    np_dt1 = ml_dtypes.float8_e4m3 if P1_DT == "fp8" else np.float16

    # ---- host gathers + normalization (tiny) ----
    ref_t = tf[:, 0, ref_perm, :]                       # [B, 256, 1024]
    ref_s = sf[:, 0, ref_perm, :]
    refn = _norm_rows(ref_t)

    # ---- phase 1: sharded cosine-sim + per-block top-8 ----
    in_maps1 = []
    for c in range(N_CORES):
        b, s = divmod(c, 4)
        xn = _norm_rows(tf[b, EXTRA_FRAMES[s]])         # [4096, 1024]
        # extP[p, eb, k, e] = xn.T[k*128+p, eb*EB+e]
        extP = np.ascontiguousarray(
            xn.T.reshape(8, 128, NBLK, EB).transpose(1, 2, 0, 3)).astype(np_dt1)
        # refP[p, k, r] = refn[b].T[k*128+p, r]
        refP = np.ascontiguousarray(
            refn[b].T.reshape(8, 128, NUM_REF).transpose(1, 0, 2)).astype(np_dt1)
        in_maps1.append({"extP": extP, "refP": refP})

    res1 = bass_utils.run_bass_kernel_spmd(
        _get("p1"), in_maps1, core_ids=list(range(N_CORES)))
    LAST_PERF["p1"] = res1

    # ---- host cross-shard top-k merge ----
    gidx = np.zeros((B, NUM_REF, TOPK), dtype=np.int64)
    for b in range(B):
        vals, idxs = [], []
        for s in range(4):
            r = res1.results[b * 4 + s]
            # [eb, m, p, 8] -> [m*128+p, eb*8]
            v = r["vals"].astype(np.float32).transpose(1, 2, 0, 3).reshape(NUM_REF, -1)
            ix = r["idx"].astype(np.int64)
            gx = (np.arange(NBLK, dtype=np.int64)[:, None, None, None] * EB
                  + ix + s * P).transpose(1, 2, 0, 3).reshape(NUM_REF, -1)
            vals.append(v)
            idxs.append(gx)
        vals = np.concatenate(vals, axis=1)
        idxs = np.concatenate(idxs, axis=1)
        order = np.argsort(-vals, axis=1, kind="stable")[:, :TOPK]
        gidx[b] = np.take_along_axis(idxs, order, axis=1)

    fr = np.asarray(EXTRA_FRAMES, dtype=np.int64)[gidx // P]
    pt = gidx % P
    sim_high = tf[np.arange(B)[:, None, None], fr, pt]  # [B, 256, 4, 1024]

    # ---- phase 2: distances ----
    sh_t = np.stack([tf[:, t, shared_perm, :] for t in SHARED_TEACHER], axis=1)
    sh_s = np.stack([sf[:, s, shared_perm, :] for s in SHARED_STUDENT], axis=1)

    in_maps2 = []
    for c in range(N_CORES):
        b, h, dh = c >> 2, (c >> 1) & 1, c & 1
        rs = slice(h * 128, (h + 1) * 128)
        cs = slice(dh * DH, (dh + 1) * DH)
        srcs = [ref_t[b, rs, cs], ref_s[b, rs, cs]]
        srcs += [sh_t[b, j, rs, cs] for j in range(3)]
        srcs += [sh_s[b, j, rs, cs] for j in range(3)]
        srcs += [sim_high[b, rs, k, cs] for k in range(4)]
        src = np.ascontiguousarray(np.stack(srcs, axis=1)).astype(np.float16)
        in_maps2.append({"src": src})

    nc2, cols, nv, ng = _get("p2")
    res2 = bass_utils.run_bass_kernel_spmd(
        nc2, in_maps2, core_ids=list(range(N_CORES)))
    LAST_PERF["p2"] = res2

    # ---- host tail: reconstruct Z, kl + SmoothL1 + averaging ----
    def z_of(core):
        r = res2.results[core]
        zv = r["zv"].astype(np.float64)
        zg = r["zg"].astype(np.float64) if ng else None
        z = np.zeros((128, N_UNITS, 3))
        for (u, c), (kind, i) in cols.items():
            z[:, u, c] = zv[:, i] if kind == "v" else zg[:, i]
        return z

    s1 = s2 = s3 = 0.0
    for b in range(B):
        for h in range(2):
            z = z_of(b * 4 + h * 2 + 0) + z_of(b * 4 + h * 2 + 1)
            Zt, Zs, num = z[..., 0], z[..., 1], z[..., 2]   # [128, 19]
            kl = num / Zt - np.log(Zt) + np.log(Zs)
            akl = np.abs(kl)
            hub = np.where(akl < BETA, 0.5 * kl * kl / BETA, akl - 0.5 * BETA)
            s1 += hub[:, 0:3].sum()
            s2 += hub[:, 3:7].sum()
            s3 += hub[:, 7:19].sum()

    loss = (s1 / (3 * B * NUM_REF)
            + s2 / (B * NUM_REF * TOPK)
            + s3 / (3 * B * NUM_REF * TOPK))
    return np.float32(loss)


# revision 10
# speedup vs baseline: 1.5572x; 1.1394x over previous
"""Trainium2 Bass kernel for nn_DA3CrossFrameCFDistanceLoss.

Strategy (8 NeuronCores):
  Phase 1 (data-parallel over batch x extra-frame shard):
    core c -> (b = c//4, shard s = c%4).  Host pre-normalizes the ref rows
    and the shard's candidate rows and quantizes both to fp8e4m3, packed
    partition-major so every DMA descriptor is a 16KB contiguous run.
    The PE computes cosine sims with DoubleRow fp8 matmuls (2 k-chunks per
    instruction), ACT copies each finished PSUM block to SBUF as fp16, and
    the DVE extracts per-2048-block top-8 values + indices, pipelined
    behind the matmuls.  Results accumulate in SBUF and ship in one DMA.
    Host merges the 4 shards x 2 blocks x 8 candidates/row to the top-4.
  Phase 2 (data-parallel over (batch, row-half, feature-half)):
    per KL unit: xt/xs subs run on DVE (fp16 tensor_tensor = 2x rate) or
    GpSimd, ACT computes exp with a fused accumulate (Zt/Zs + the et/es
    tensors in one op), and num = sum(et*dap) runs either as a fused
    DVE scalar_tensor_tensor or split as DVE-mult + ACT identity-accum,
    balancing DVE against ACT.  Host combines the feature-half partials,
    evaluates kl = num/Zt - log Zt + log Zs, SmoothL1, and the averaging.
"""

import os

import numpy as np
import ml_dtypes

import concourse.bass as bass
from concourse import bacc
import concourse.mybir as mybir
from concourse import bass_utils
from concourse.tile import TileContext

# ---- problem constants (hardcoded from the nn.Module defaults) ----
B, V, P, D = 2, 8, 4096, 1024
EXTRA_FRAMES = [1, 3, 5, 7]
SHARED_TEACHER = [2, 4, 6]
SHARED_STUDENT = [1, 2, 3]
NUM_REF = 256
NUM_SHARED = 256
TOPK = 4
BETA = 0.5
N_CORES = 8

EB = 2048                 # phase-1 e-block size
NBLK = P // EB            # blocks per shard
DH = D // 2               # phase-2 feature half
N_UNITS = 19              # 3 d1 + 4 d2 + 12 d3

P1_DT = os.environ.get("BASS_P1_DT", "fp8")     # "fp8" | "fp16"
P1_DUMMY = int(os.environ.get("BASS_P1_DUMMY", "0"))
N_STT = int(os.environ.get("BASS_P2_NSTT", "12"))  # nums fused on DVE
GS = int(os.environ.get("BASS_P2_GS", "8"))        # subs on gpsimd

F32 = mybir.dt.float32
F16 = mybir.dt.float16
F8 = mybir.dt.float8e4
U16 = mybir.dt.uint16

_CACHE = {}

# Results of the most recent launches (exec_time_ns etc), for test harnesses.
LAST_PERF = {}


def _build_phase1():
    DT = F8 if P1_DT == "fp8" else F16
    nc = bacc.Bacc("TRN2", target_bir_lowering=False, debug=False,
                   enable_asserts=False, num_devices=N_CORES)
    refP = nc.dram_tensor("refP", (128, 8, NUM_REF), DT, kind="ExternalInput").ap()
    extP = nc.dram_tensor("extP", (128, NBLK, 8, EB), DT, kind="ExternalInput").ap()
    vals_o = nc.dram_tensor("vals", (128, NBLK, 2, 8), F16, kind="ExternalOutput").ap()
    idx_o = nc.dram_tensor("idx", (128, NBLK, 2, 8), U16, kind="ExternalOutput").ap()

    DR = mybir.MatmulPerfMode.DoubleRow

    with TileContext(nc) as tc:
        with (
            tc.tile_pool(name="const", bufs=1) as cpool,
            tc.tile_pool(name="xin", bufs=2) as xpool,
            tc.tile_pool(name="sim", bufs=2) as spool,
            tc.tile_pool(name="ps", bufs=2, space="PSUM") as pspool,
        ):
            ref_sb = cpool.tile([128, 8, NUM_REF], DT)
            nc.sync.dma_start(out=ref_sb, in_=refP)
            vals_sb = cpool.tile([128, NBLK, 2, 8], F16)
            idx_sb = cpool.tile([128, NBLK, 2, 8], U16)
            for eb in range(NBLK):
                xt = xpool.tile([128, 8, EB], DT, tag="xt")
                nc.sync.dma_start(out=xt, in_=extP[:, eb])
                for m in range(2):
                    ps = pspool.tile([128, EB], F32, tag="ps", name="ps")
                    if m == 0 and P1_DUMMY:
                        # throwaway matmul absorbs the xt-DMA wait (walrus
                        # S3_LW limit: one semaphore wait per PE matmul)
                        nc.tensor.matmul(ps[:, 0:512], lhsT=xt[:, 0, 0:128],
                                         rhs=xt[:, 0, 0:512],
                                         start=True, stop=True,
                                         skip_group_check=True)
                    msl = slice(m * 128, (m + 1) * 128)
                    if DT == F8:
                        for kk in range(4):
                            for nn in range(EB // 512):
                                nc.tensor.matmul(
                                    ps[:, nn * 512:(nn + 1) * 512],
                                    lhsT=ref_sb[:, 2 * kk:2 * kk + 2, msl],
                                    rhs=xt[:, 2 * kk:2 * kk + 2,
                                           nn * 512:(nn + 1) * 512],
                                    start=(kk == 0), stop=(kk == 3),
                                    perf_mode=DR,
                                )
                    else:
                        for k in range(8):
                            for nn in range(EB // 512):
                                nc.tensor.matmul(
                                    ps[:, nn * 512:(nn + 1) * 512],
                                    lhsT=ref_sb[:, k, msl],
                                    rhs=xt[:, k, nn * 512:(nn + 1) * 512],
                                    start=(k == 0), stop=(k == 7),
                                )
                    sim = spool.tile([128, EB], F16, tag="sim", name="sim")
                    nc.scalar.copy(sim, ps)
                    nc.vector.max(vals_sb[:, eb, m, :], sim)
                    nc.vector.max_index(idx_sb[:, eb, m, :],
                                        vals_sb[:, eb, m, :], sim)
            nc.sync.dma_start(out=vals_o, in_=vals_sb)
            nc.sync.dma_start(out=idx_o, in_=idx_sb)
    nc.compile()
    return nc


def _p2_unit_order():
    """(u, kind, j, k); u is the reference unit index
    (d1 j -> u=j, d2 k -> u=3+k, d3 (j,k) -> u=7+4j+k)."""
    order = [(3 + k, "d2", None, k) for k in range(4)]
    order += [(7 + 4 * j + k, "d3", j, k) for j in range(3) for k in range(4)]
    order += [(j, "d1", j, None) for j in range(3)]
    return order


def _p2_plan(n_stt, gs):
    """Static schedule: which engine runs each piece.
    Returns (plan, na, nd) where plan maps (u, c) -> (acc_tile, col):
    acc_tile 'a' = ACT-written accumulator, 'd' = DVE-written.
    Also assigns: first `gs` emitted d3 xs-subs -> gpsimd;
    last `n_stt` emitted nums -> fused DVE stt."""
    plan = {}
    ai = di = 0
    order = _p2_unit_order()
    nums_fused = {order[len(order) - 1 - i][0] for i in range(min(n_stt, 19))}
    gp_subs = set()
    cnt = 0
    for u, kind, j, k in order:
        if kind == "d3" and cnt < gs:
            gp_subs.add(u)
            cnt += 1
    for u, kind, j, k in order:
        plan[(u, 0)] = ("a", ai); ai += 1
        plan[(u, 1)] = ("a", ai); ai += 1
        if u in nums_fused:
            plan[(u, 2)] = ("d", di); di += 1
        else:
            plan[(u, 2)] = ("a", ai); ai += 1
    return plan, ai, di, nums_fused, gp_subs


def _build_phase2():
    plan, na, nd, nums_fused, gp_subs = _p2_plan(N_STT, GS)
    nc = bacc.Bacc("TRN2", target_bir_lowering=False, debug=False,
                   enable_asserts=False, num_devices=N_CORES)
    SRC = nc.dram_tensor("src", (128, 12, DH), F16, kind="ExternalInput").ap()
    ZA = nc.dram_tensor("za", (128, na), F32, kind="ExternalOutput").ap()
    ZD = (nc.dram_tensor("zd", (128, nd), F32, kind="ExternalOutput").ap()
          if nd else None)

    Exp = mybir.ActivationFunctionType.Exp
    Ident = mybir.ActivationFunctionType.Identity
    mult = mybir.AluOpType.mult

    with TileContext(nc) as tc:
        with tc.tile_pool(name="main", bufs=1) as pool:
            src = pool.tile([128, 12, DH], F16)
            nc.sync.dma_start(out=src, in_=SRC)
            # src slots: 0=ref_t 1=ref_s 2..4=sht_j 5..7=shs_j 8..11=simh_k
            rd = pool.tile([128, DH], F16)
            sd = pool.tile([128, 3, DH], F16)
            dd1 = pool.tile([128, 3, DH], F16)
            xts = pool.tile([128, 4, DH], F16)   # rotating xt slots
            xss = pool.tile([128, 4, DH], F16)   # rotating xs slots
            ets = pool.tile([128, 4, DH], F16)   # rotating et slots
            ess = pool.tile([128, 4, DH], F16)   # rotating es slots (unused val)
            ws = pool.tile([128, 2, DH], F16)    # rotating num-product slots
            za = pool.tile([128, na], F32)
            zd = pool.tile([128, nd], F32, name="zd") if nd else None

            nc.vector.tensor_sub(rd, src[:, 0, :], src[:, 1, :])
            nc.vector.tensor_sub(sd, src[:, 2:5, :], src[:, 5:8, :])
            for j in range(3):
                nc.vector.tensor_sub(dd1[:, j, :], rd, sd[:, j, :])

            def dap_of(kind, j):
                return rd if kind == "d2" else (sd[:, j, :] if kind == "d3"
                                                else dd1[:, j, :])

            for i, (u, kind, j, k) in enumerate(_p2_unit_order()):
                if kind == "d2":
                    at, bt = src[:, 0, :], src[:, 8 + k, :]
                    as_, bs = src[:, 1, :], src[:, 8 + k, :]
                elif kind == "d3":
                    at, bt = src[:, 2 + j, :], src[:, 8 + k, :]
                    as_, bs = src[:, 5 + j, :], src[:, 8 + k, :]
                else:
                    at, bt = src[:, 0, :], src[:, 2 + j, :]
                    as_, bs = src[:, 1, :], src[:, 5 + j, :]
                xt = xts[:, i % 4, :]
                xs = xss[:, i % 4, :]
                et = ets[:, i % 4, :]
                es = ess[:, i % 4, :]
                nc.vector.tensor_sub(xt, at, bt)
                if u in gp_subs:
                    nc.gpsimd.tensor_sub(xs, as_, bs)
                else:
                    nc.vector.tensor_sub(xs, as_, bs)
                _, ca0 = plan[(u, 0)]
                nc.scalar.activation(et, xt, Exp, accum_out=za[:, ca0:ca0 + 1])
                _, ca1 = plan[(u, 1)]
                nc.scalar.activation(es, xs, Exp, accum_out=za[:, ca1:ca1 + 1])
                kindc, c2 = plan[(u, 2)]
                if kindc == "d":
                    nc.vector.scalar_tensor_tensor(
                        out=ws[:, 0, :], in0=et, scalar=1.0,
                        in1=dap_of(kind, j), op0=mult, op1=mult,
                        accum_out=zd[:, c2:c2 + 1])
                else:
                    w = ws[:, 1, :]
                    nc.vector.tensor_mul(w, et, dap_of(kind, j))
                    nc.scalar.activation(w, w, Ident,
                                         accum_out=za[:, c2:c2 + 1])

            nc.sync.dma_start(out=ZA, in_=za)
            if nd:
                nc.sync.dma_start(out=ZD, in_=zd)
    nc.compile()
    return nc, plan, na, nd


def _get(name):
    if name not in _CACHE:
        _CACHE[name] = _build_phase1() if name == "p1" else _build_phase2()
    return _CACHE[name]


def _norm_rows(x):
    n = np.sqrt(np.einsum("...d,...d->...", x, x))
    return x / np.maximum(n, 1e-12)[..., None]


def kernel(**inputs):
    tf = np.ascontiguousarray(np.asarray(inputs["teacher_feats"], dtype=np.float32))
    sf = np.ascontiguousarray(np.asarray(inputs["student_feats"], dtype=np.float32))
    in_dtype = np.asarray(inputs["ref_perm"]).dtype
    ref_perm = np.asarray(inputs["ref_perm"]).astype(np.int64)[:NUM_REF]
    shared_perm = np.asarray(inputs["shared_perm"]).astype(np.int64)[:NUM_SHARED]
    assert in_dtype == np.int32

    np_dt1 = ml_dtypes.float8_e4m3 if P1_DT == "fp8" else np.float16

    # ---- host gathers + normalization (tiny) ----
    ref_t = tf[:, 0, ref_perm, :]                       # [B, 256, 1024]
    ref_s = sf[:, 0, ref_perm, :]
    refn = _norm_rows(ref_t)

    # ---- phase 1: sharded cosine-sim + per-block top-8 ----
    in_maps1 = []
    for c in range(N_CORES):
        b, s = divmod(c, 4)
        xn = _norm_rows(tf[b, EXTRA_FRAMES[s]])         # [4096, 1024]
        # extP[p, eb, k, e] = xn.T[k*128+p, eb*EB+e]
        extP = np.ascontiguousarray(
            xn.T.reshape(8, 128, NBLK, EB).transpose(1, 2, 0, 3)).astype(np_dt1)
        # refP[p, k, r] = refn[b].T[k*128+p, r]
        refP = np.ascontiguousarray(
            refn[b].T.reshape(8, 128, NUM_REF).transpose(1, 0, 2)).astype(np_dt1)
        in_maps1.append({"extP": extP, "refP": refP})

    res1 = bass_utils.run_bass_kernel_spmd(
        _get("p1"), in_maps1, core_ids=list(range(N_CORES)))
    LAST_PERF["p1"] = res1

    # ---- host cross-shard top-k merge ----
    gidx = np.zeros((B, NUM_REF, TOPK), dtype=np.int64)
    for b in range(B):
        vals, idxs = [], []
        for s in range(4):
            r = res1.results[b * 4 + s]
            # [p, eb, m, 8] -> [m*128+p, eb*8]
            v = r["vals"].astype(np.float32).transpose(2, 0, 1, 3).reshape(NUM_REF, -1)
            ix = r["idx"].astype(np.int64)
            gx = (np.arange(NBLK, dtype=np.int64)[None, :, None, None] * EB
                  + ix + s * P).transpose(2, 0, 1, 3).reshape(NUM_REF, -1)
            vals.append(v)
            idxs.append(gx)
        vals = np.concatenate(vals, axis=1)
        idxs = np.concatenate(idxs, axis=1)
        order = np.argsort(-vals, axis=1, kind="stable")[:, :TOPK]
        gidx[b] = np.take_along_axis(idxs, order, axis=1)

    fr = np.asarray(EXTRA_FRAMES, dtype=np.int64)[gidx // P]
    pt = gidx % P
    sim_high = tf[np.arange(B)[:, None, None], fr, pt]  # [B, 256, 4, 1024]

    # ---- phase 2: distances ----
    sh_t = np.stack([tf[:, t, shared_perm, :] for t in SHARED_TEACHER], axis=1)
    sh_s = np.stack([sf[:, s, shared_perm, :] for s in SHARED_STUDENT], axis=1)

    in_maps2 = []
    for c in range(N_CORES):
        b, h, dh = c >> 2, (c >> 1) & 1, c & 1
        rs = slice(h * 128, (h + 1) * 128)
        cs = slice(dh * DH, (dh + 1) * DH)
        srcs = [ref_t[b, rs, cs], ref_s[b, rs, cs]]
        srcs += [sh_t[b, j, rs, cs] for j in range(3)]
        srcs += [sh_s[b, j, rs, cs] for j in range(3)]
        srcs += [sim_high[b, rs, k, cs] for k in range(4)]
        src = np.ascontiguousarray(np.stack(srcs, axis=1)).astype(np.float16)
        in_maps2.append({"src": src})

    nc2, plan, na, nd = _get("p2")
    res2 = bass_utils.run_bass_kernel_spmd(
        nc2, in_maps2, core_ids=list(range(N_CORES)))
    LAST_PERF["p2"] = res2

    # ---- host tail: reconstruct Z, kl + SmoothL1 + averaging ----
    def z_of(core):
        r = res2.results[core]
        za = r["za"].astype(np.float64)
        zdv = r["zd"].astype(np.float64) if nd else None
        z = np.zeros((128, N_UNITS, 3))
        for (u, c), (kind, i) in plan.items():
            z[:, u, c] = za[:, i] if kind == "a" else zdv[:, i]
        return z

    s1 = s2 = s3 = 0.0
    for b in range(B):
        for h in range(2):
            z = z_of(b * 4 + h * 2 + 0) + z_of(b * 4 + h * 2 + 1)
            Zt, Zs, num = z[..., 0], z[..., 1], z[..., 2]   # [128, 19]
            kl = num / Zt - np.log(Zt) + np.log(Zs)
            akl = np.abs(kl)
            hub = np.where(akl < BETA, 0.5 * kl * kl / BETA, akl - 0.5 * BETA)
            s1 += hub[:, 0:3].sum()
            s2 += hub[:, 3:7].sum()
            s3 += hub[:, 7:19].sum()

    loss = (s1 / (3 * B * NUM_REF)
            + s2 / (B * NUM_REF * TOPK)
            + s3 / (3 * B * NUM_REF * TOPK))
    return np.float32(loss)
